# revision 1
# baseline (speedup 1.0000x reference)
"""GAT layer kernel for 8 Trainium2 NeuronCores.

Math (per core, rows i in its 512-row slice, j = all 4096 nodes):
  g = x @ W1 -> [N, H, F];  el/er = head-wise projections of g on attn_l/attn_r
  e_ij = leaky_relu(el_i + er_j, 0.2); masked by adj; softmax over j; aggregate.

Key identity used on-chip: exp(lrelu(s)) = max(e^s, e^{0.2 s}).  Factoring the
per-row constant e^{0.2 el_i} (cancels in the softmax) gives attention weights
  B[j, i] = adj[i, j] * max(R_i * Er_j, Er5_j)
with R = e^{0.8 el}, Er = e^{er}, Er5 = e^{0.2 er}.  So the N^2 x H map needs no
per-element transcendentals: one fused tensor_scalar (mult+max) and one mask
multiply per element, then TensorE matmuls aggregate numerator and denominator.

Layout: everything runs transposed ([feature/j on partitions, i on free]).
Adjacency arrives via gpsimd cast-DMA (int32->fp16) in natural row layout, is
transposed on TensorE into PSUM, and the mask multiply reads it straight from
PSUM.  The final output is produced as out^T (host transposes back).
"""

import numpy as np

N = 4096
IN_F = 128
H = 4
F = 64
NH = H * F  # 256
OUT = 128
NCORES = 8
ROWS = N // NCORES  # 512 rows per core
JT = N // 128  # 32 j-tiles
GBLK = H * (F + 1)  # 260: g block per j-tile (64 feats + ones col per head)

_CACHE = {}


def _build(reps=1, loop_n=None, tt_merge=False, tt_perhead=False, deep=3):
    import concourse.bass as bass
    import concourse.tile as tile
    from concourse import bacc, mybir
    from concourse.masks import make_identity
    from contextlib import ExitStack

    dt = mybir.dt
    Alu = mybir.AluOpType
    Act = mybir.ActivationFunctionType

    nc = bacc.Bacc("TRN2", target_bir_lowering=False, debug=False)

    xT_d = nc.dram_tensor("xT", [IN_F, N], dt.float16, kind="ExternalInput").ap()
    sw_d = nc.dram_tensor("sw", [IN_F, ROWS + 2 * H + NH], dt.float16, kind="ExternalInput").ap()
    wout_d = nc.dram_tensor("wout", [F, H, OUT], dt.float16, kind="ExternalInput").ap()
    bout_d = nc.dram_tensor("bout", [1, OUT], dt.float16, kind="ExternalInput").ap()
    adj_d = nc.dram_tensor("adj", [ROWS, N], dt.int32, kind="ExternalInput").ap()
    out_d = nc.dram_tensor("outT", [OUT, ROWS], dt.float32, kind="ExternalOutput").ap()

    NCHUNK = 4  # j-quarters per i-block for the adjacency cast-DMA
    CW = N // NCHUNK  # 1024 columns per chunk
    NG = 4  # er psum groups
    GJT = JT // NG  # 8 j-tiles per er group

    with tile.TileContext(nc) as tc:
        with ExitStack() as ctx:
            singles = ctx.enter_context(tc.tile_pool(name="singles", bufs=1))
            psum_acc = ctx.enter_context(tc.tile_pool(name="pacc", bufs=1, space="PSUM"))
            psum_g = ctx.enter_context(tc.tile_pool(name="pg_pool", bufs=1, space="PSUM"))
            psum_t = ctx.enter_context(tc.tile_pool(name="pt_pool", bufs=2, space="PSUM"))
            psum_er = ctx.enter_context(tc.tile_pool(name="per_pool", bufs=1, space="PSUM"))
            q_pool = ctx.enter_context(tc.tile_pool(name="qp", bufs=deep))
            b_pool = ctx.enter_context(tc.tile_pool(name="bp", bufs=deep))
            ep_pool = ctx.enter_context(tc.tile_pool(name="epp", bufs=1))

            # ---- constants first: Pool executes in order, so these must
            # precede the adj cast-DMA descriptor generation ----
            ident = singles.tile([128, 128], dt.float16)
            make_identity(nc, ident)
            ones_row = singles.tile([1, ROWS], dt.float16)
            nc.gpsimd.memset(ones_row, 1.0)
            ones_col = singles.tile([1, 128], dt.float16)
            nc.gpsimd.memset(ones_col, 1.0)
            onesH16 = singles.tile([128, H], dt.float16)
            nc.gpsimd.memset(onesH16, 1.0)

            # ---- one-time loads ----
            sw = singles.tile([IN_F, ROWS + 2 * H + NH], dt.float16)
            nc.sync.dma_start(sw, sw_d)
            xTo = sw[:, 0:ROWS]
            wr = sw[:, ROWS : ROWS + H]
            wl = sw[:, ROWS + H : ROWS + 2 * H]
            w1 = sw[:, ROWS + 2 * H : ROWS + 2 * H + NH]
            xT = singles.tile([IN_F, N], dt.float16)
            for xc in range(4):
                nc.sync.dma_start(
                    xT[:, (N // 4) * xc : (N // 4) * (xc + 1)],
                    xT_d[:, (N // 4) * xc : (N // 4) * (xc + 1)],
                )
            wout = singles.tile([F, H, OUT], dt.float16)
            nc.sync.dma_start(wout, wout_d)
            bout = singles.tile([1, OUT], dt.float16)
            nc.sync.dma_start(bout, bout_d)
            ones128_32 = singles.tile([128, 1], dt.float32)
            nc.gpsimd.memset(ones128_32, 1.0)
            ident32 = singles.tile([128, 128], dt.float32)
            make_identity(nc, ident32)

            def rep_body(rep):
                # ---- adjacency cast-DMA: int32 -> fp16, [i, j] layout ----
                adjf = {}
                for jc in range(NCHUNK):
                    for b in range(4):
                        t = singles.tile(
                            [128, CW], dt.float16, name=f"adjf_{b}_{jc}_{rep}",
                            tag=f"adjf_{b}_{jc}",
                        )
                        nc.gpsimd.dma_start(
                            t, adj_d[128 * b : 128 * (b + 1), CW * jc : CW * (jc + 1)]
                        )
                        adjf[(b, jc)] = t

                # ---- own-row head projections: R = exp(0.8 * el), broadcast ----
                r_bc = []
                for h in range(H):
                    hp_pool, hp_tag = (psum_g, "pg") if h % 2 == 0 else (psum_t, "pT")
                    pel = hp_pool.tile([1, ROWS], dt.float32, tag=hp_tag, name=f"pel{h}_{rep}")
                    nc.tensor.matmul(
                        pel, lhsT=wl[:, h : h + 1], rhs=xTo, start=True, stop=True
                    )
                    r_row = ep_pool.tile([1, ROWS], dt.float16, tag=f"r_row{h % 2}",
                                         name=f"r_row{h}_{rep}")
                    nc.scalar.activation(r_row, pel, Act.Exp, scale=0.8)
                    pbc = hp_pool.tile([128, ROWS], dt.float32, tag=hp_tag, name=f"pbc{h}_{rep}")
                    nc.tensor.matmul(pbc, lhsT=ones_col, rhs=r_row, start=True, stop=True)
                    rb = singles.tile([128, ROWS], dt.float16, name=f"r_bc{h}_{rep}",
                                      tag=f"r_bc{h}")
                    nc.scalar.copy(rb, pbc)
                    r_bc.append(rb)

                # ---- er head projections (packed psum groups) + exp ----
                er_g, er5_g = [], []
                for grp in range(NG):
                    per = psum_acc.tile(
                        [128, H * GJT], dt.float32, tag=f"acc{grp}", name=f"per{grp}_{rep}"
                    )
                    for k in range(GJT):
                        jt = GJT * grp + k
                        nc.tensor.matmul(
                            per[:, H * k : H * (k + 1)],
                            lhsT=xT[:, 128 * jt : 128 * (jt + 1)],
                            rhs=wr,
                            start=True,
                            stop=True,
                        )
                    e1 = singles.tile([128, H * GJT], dt.float32, name=f"er_{grp}_{rep}",
                                      tag=f"er_{grp}")
                    nc.scalar.activation(e1, per, Act.Exp)
                    e5 = singles.tile([128, H * GJT], dt.float32, name=f"er5_{grp}_{rep}",
                                      tag=f"er5_{grp}")
                    nc.scalar.activation(e5, per, Act.Exp, scale=0.2)
                    er_g.append(e1)
                    er5_g.append(e5)

                # ---- projection g = x @ W1 (per j-tile tiles for dep granularity) ----
                g_t = []
                for jt in range(JT):
                    pg = psum_g.tile([128, NH], dt.float32, tag="pg", name=f"pg{jt}_{rep}")
                    nc.tensor.matmul(
                        pg,
                        lhsT=xT[:, 128 * jt : 128 * (jt + 1)],
                        rhs=w1,
                        start=True,
                        stop=True,
                    )
                    gt = singles.tile([128, GBLK], dt.float16, name=f"g_{jt}_{rep}",
                                      tag=f"g_{jt}")
                    gt3 = gt.rearrange("p (h f) -> p h f", h=H)
                    nc.scalar.copy(
                        gt3[:, :, 0:F], pg.rearrange("p (h f) -> p h f", h=H)
                    )
                    nc.scalar.copy(gt3[:, :, F : F + 1], onesH16.unsqueeze(2))
                    g_t.append(gt)

                # ---- attention accumulation over j-tiles ----
                pacc = [
                    psum_acc.tile([F + 1, ROWS], dt.float32, name=f"acc{h}_{rep}", tag=f"acc{h}")
                    for h in range(H)
                ]
                if not tt_merge:
                    for jt in range(JT):
                        jc, jcol = jt // (JT // NCHUNK), 128 * (jt % (JT // NCHUNK))
                        grp, gk = jt // GJT, jt % GJT
                        pT = psum_t.tile([128, ROWS], dt.float16, tag="pT", name=f"pT{jt}_{rep}")
                        for b in range(4):
                            nc.tensor.transpose(
                                pT[:, 128 * b : 128 * (b + 1)],
                                adjf[(b, jc)][:, jcol : jcol + 128],
                                ident,
                            )
                        q2 = q_pool.tile([128, H * ROWS], dt.float16, tag="q2")
                        for h in range(H):
                            nc.vector.tensor_scalar(
                                q2[:, ROWS * h : ROWS * (h + 1)],
                                r_bc[h],
                                er_g[grp][:, H * gk + h : H * gk + h + 1],
                                er5_g[grp][:, H * gk + h : H * gk + h + 1],
                                Alu.mult,
                                Alu.max,
                            )
                        ball = b_pool.tile([128, H * ROWS], dt.float16, tag="ball")
                        if tt_perhead:
                            for h in range(H):
                                nc.vector.tensor_tensor(
                                    ball[:, ROWS * h : ROWS * (h + 1)],
                                    q2[:, ROWS * h : ROWS * (h + 1)],
                                    pT,
                                    Alu.mult,
                                )
                        else:
                            adj_rep = bass.AP(
                                tensor=pT.tensor,
                                offset=pT.offset,
                                ap=[pT.ap[0], [0, H], [1, ROWS]],
                            )
                            nc.vector.tensor_tensor(ball, q2, adj_rep, Alu.mult)
                        for h in range(H):
                            nc.tensor.matmul(
                                pacc[h],
                                lhsT=g_t[jt][:, (F + 1) * h : (F + 1) * (h + 1)],
                                rhs=ball[:, ROWS * h : ROWS * (h + 1)],
                                start=(jt == 0),
                                stop=(jt == JT - 1),
                            )
                for jp in range(JT // 2 if tt_merge else 0):
                    # two j-tiles per round: one [128, 1024] psum (1 bank), one
                    # FD=4096 mask-TT, 8 accumulating matmuls.
                    pT2 = psum_t.tile([128, 2 * ROWS], dt.float16, tag="pT",
                                      name=f"pT{jp}_{rep}")
                    q2 = q_pool.tile([128, 2 * H * ROWS], dt.float16, tag="q2")
                    for t in range(2):
                        jt = 2 * jp + t
                        jc, jcol = jt // (JT // NCHUNK), 128 * (jt % (JT // NCHUNK))
                        grp, gk = jt // GJT, jt % GJT
                        for b in range(4):
                            nc.tensor.transpose(
                                pT2[:, ROWS * t + 128 * b : ROWS * t + 128 * (b + 1)],
                                adjf[(b, jc)][:, jcol : jcol + 128],
                                ident,
                            )
                        for h in range(H):
                            nc.vector.tensor_scalar(
                                q2[:, H * ROWS * t + ROWS * h : H * ROWS * t + ROWS * (h + 1)],
                                r_bc[h],
                                er_g[grp][:, H * gk + h : H * gk + h + 1],
                                er5_g[grp][:, H * gk + h : H * gk + h + 1],
                                Alu.mult,
                                Alu.max,
                            )
                    ball = b_pool.tile([128, 2 * H * ROWS], dt.float16, tag="ball")
                    adj_rep = bass.AP(
                        tensor=pT2.tensor,
                        offset=pT2.offset,
                        ap=[pT2.ap[0], [ROWS, 2], [0, H], [1, ROWS]],
                    )
                    nc.vector.tensor_tensor(ball, q2, adj_rep, Alu.mult)
                    for t in range(2):
                        jt = 2 * jp + t
                        for h in range(H):
                            nc.tensor.matmul(
                                pacc[h],
                                lhsT=g_t[jt][:, (F + 1) * h : (F + 1) * (h + 1)],
                                rhs=ball[:, H * ROWS * t + ROWS * h : H * ROWS * t + ROWS * (h + 1)],
                                start=(jt == 0),
                                stop=(jt == JT - 1),
                            )

                # ---- epilogue: reciprocal of denominators via transpose trick ----
                den64 = ep_pool.tile([65, H * ROWS], dt.float32, tag="den64")
                for h in range(H):
                    if h % 2 == 0:
                        nc.scalar.copy(
                            den64[F : F + 1, ROWS * h : ROWS * (h + 1)], pacc[h][F : F + 1, :]
                        )
                    else:
                        nc.vector.tensor_copy(
                            den64[F : F + 1, ROWS * h : ROWS * (h + 1)], pacc[h][F : F + 1, :]
                        )
                NBLK = H * ROWS // 128  # 16
                denT_p = psum_t.tile([128, NBLK], dt.float32, tag="pT", name=f"denT_p_{rep}")
                for k in range(NBLK):
                    nc.tensor.matmul(
                        denT_p[:, k : k + 1],
                        lhsT=den64[F : F + 1, 128 * k : 128 * (k + 1)],
                        rhs=ones128_32[F : F + 1, :],
                        start=True,
                        stop=True,
                    )
                denT = ep_pool.tile([128, NBLK], dt.float32, tag="denT")
                nc.scalar.copy(denT, denT_p)
                recT = ep_pool.tile([128, NBLK], dt.float32, tag="recT")
                nc.vector.reciprocal(recT, denT)
                rec_all = ep_pool.tile([1, H * ROWS], dt.float16, tag="rec_all")
                for h in range(H):
                    rp_pool, rp_tag = (psum_g, "pg") if h % 2 == 0 else (psum_t, "pT")
                    rec_p = rp_pool.tile([1, ROWS], dt.float32, tag=rp_tag, name=f"rec_p{h}_{rep}")
                    for b in range(4):
                        nc.tensor.transpose(
                            rec_p[:, 128 * b : 128 * (b + 1)],
                            recT[:, 4 * h + b : 4 * h + b + 1],
                            ident32,
                        )
                    if h % 2 == 0:
                        nc.scalar.copy(rec_all[:, ROWS * h : ROWS * (h + 1)], rec_p)
                    else:
                        nc.vector.tensor_copy(rec_all[:, ROWS * h : ROWS * (h + 1)], rec_p)

                # ---- divide, elu' = relu(x) + exp(min(x, 0)), output proj ----
                pout = psum_er.tile([OUT, ROWS], dt.float32, tag="per_out", name=f"pout_{rep}")
                for h in range(H):
                    rb_pool, rb_tag = (psum_g, "pg") if h % 2 == 0 else (psum_t, "pT")
                    rbp = rb_pool.tile([F, ROWS], dt.float32, tag=rb_tag, name=f"rbp{h}_{rep}")
                    nc.tensor.matmul(
                        rbp,
                        lhsT=ones_col[:, 0:F],
                        rhs=rec_all[:, ROWS * h : ROWS * (h + 1)],
                        start=True,
                        stop=True,
                    )
                    rb = ep_pool.tile([F, ROWS], dt.float32, tag=f"rb{h % 2}", name=f"rb{h}_{rep}")
                    if h % 2 == 0:
                        nc.scalar.copy(rb, rbp)
                    else:
                        nc.vector.tensor_copy(rb, rbp)
                    # elu'(x) = max(x,0) + exp(min(x,0)) with x = numer*rb; since
                    # rb > 0, min/max commute with the multiply -> fuse on PSUM.
                    tmin = ep_pool.tile([F, ROWS], dt.float16, tag=f"tmin{h % 2}", name=f"tmin{h}_{rep}")
                    nc.vector.scalar_tensor_tensor(
                        tmin, pacc[h][0:F, :], 0.0, rb, Alu.min, Alu.mult
                    )
                    texp = ep_pool.tile([F, ROWS], dt.float16, tag=f"texp{h % 2}", name=f"texp{h}_{rep}")
                    nc.scalar.activation(texp, tmin, Act.Exp)
                    elup = ep_pool.tile([F, ROWS], dt.float16, tag=f"elup{h % 2}", name=f"elup{h}_{rep}")
                    nc.vector.scalar_tensor_tensor(
                        elup, pacc[h][0:F, :], 0.0, rb, Alu.max, Alu.mult
                    )
                    eluh = ep_pool.tile([F, ROWS], dt.float16, tag=f"eluh{h}", name=f"eluh{h}_{rep}")
                    nc.vector.tensor_tensor(eluh, elup, texp, Alu.add)
                    nc.tensor.matmul(
                        pout, lhsT=wout[:, h, :], rhs=eluh, start=(h == 0), stop=False
                    )
                nc.tensor.matmul(pout, lhsT=bout, rhs=ones_row, start=False, stop=True)
                osb = ep_pool.tile([OUT, ROWS], dt.float32, tag="osb")
                nc.scalar.copy(osb, pout)
                nc.sync.dma_start(out_d, osb)

            if loop_n is not None:
                import os as _os
                _sr = _os.environ.get("STAG_RESET", "0") == "1"
                with tc.For_i(0, loop_n, 1, staggered_reset=_sr):
                    rep_body(0)
            else:
                for rep in range(reps):
                    rep_body(rep)

    nc.compile()
    return nc


def _prep_inputs(x, adj_mat, W1, attn_l, attn_r, W_out, b_out):
    x = np.asarray(x, dtype=np.float32)
    W1 = np.asarray(W1, dtype=np.float32)
    attn_l = np.asarray(attn_l, dtype=np.float32)
    attn_r = np.asarray(attn_r, dtype=np.float32)
    W_out = np.asarray(W_out, dtype=np.float32)
    b_out = np.asarray(b_out, dtype=np.float32)
    adj = np.asarray(adj_mat).reshape(N, N)

    xT = np.ascontiguousarray(x.T).astype(np.float16)  # [128, 4096]
    W1h = W1.reshape(IN_F, H, F)
    wr = np.einsum("ihf,f->ih", W1h, attn_r).astype(np.float16)  # [128, 4]
    wl = np.einsum("ihf,f->ih", W1h, attn_l).astype(np.float16)  # [128, 4]
    w1_16 = W1.astype(np.float16)
    wout16 = np.ascontiguousarray(W_out.reshape(H, F, OUT).transpose(1, 0, 2)).astype(
        np.float16
    )
    beff = (b_out - W_out.sum(axis=0)).astype(np.float16).reshape(1, OUT)

    in_maps = []
    for c in range(NCORES):
        rows = slice(c * ROWS, (c + 1) * ROWS)
        sw = np.concatenate([xT[:, rows], wr, wl, w1_16], axis=1)
        in_maps.append(
            {
                "xT": xT,
                "sw": np.ascontiguousarray(sw),
                "wout": wout16,
                "bout": beff,
                "adj": np.ascontiguousarray(adj[rows].astype(np.int32, copy=False)),
            }
        )
    return in_maps


def kernel(**inputs):
    from concourse import bass_utils

    if "nc" not in _CACHE:
        _CACHE["nc"] = _build()
    nc = _CACHE["nc"]
    in_maps = _prep_inputs(**inputs)
    res = bass_utils.run_bass_kernel_spmd(nc, in_maps, core_ids=list(range(NCORES)))
    out = np.concatenate([res.results[c]["outT"].T for c in range(NCORES)], axis=0)
    return out.astype(np.float32)

